# revision 24
# baseline (speedup 1.0000x reference)
"""AFWM correlation->convs->warp kernel on 8 Trainium2 NeuronCores (v2).

Pure data-parallel: batch sample b -> core b. Per core:
  corr = lrelu(7x7 cost volume(feat1, feat2) / C)      [49, H, W]
  h1..h3 = lrelu(conv3x3(...)), flow = conv3x3(h3)     [2, H, W]
  out = bilinear border-clamped warp of feat2 by flow  [C, H, W]

v2 design (vs v1 baseline):
- Correlation: per row one Gram pair (510+204 free, 2D rhs AP over the
  7-row window), PSUM->SBUF copy, one diagonal-stride band DMA, PE
  transpose, fused Lrelu(x/C) write. No x-replica of corr.
- Convs: row-aligned 490-col chunks; taps as accumulating matmuls with
  K-packed replicas (h2 x2, h3 x3); Lrelu+bias fused into the single
  PSUM->SBUF activation per replica; pads never written (2D skip-pad
  APs), so the initial memset provides padding forever.
- Warp: runs on the TensorEngine. Per source row yp, an xbar
  DMA-transpose gives mst[x, c]; banded 101x96 interpolation matrices
  S_ty (bilinear coeffs on 3 diagonals, built by diagonal-pattern DMAs
  from the coefficient planes) turn the 9-tap bilinear blend into 6
  accumulating matmuls producing the output row directly in [c, x]
  layout. No DVE in the inner loop except PSUM->staging copies.
- All matmuls bf16 with fp32 PSUM accumulation.
"""

import sys
import time

import numpy as np

sys.path.insert(0, "/opt/trn_rl_repo")

NEG = 0.1
H, W, C = 128, 96, 256
CH = 2
WP = 98            # W + 2 (conv pad)
HP = 130           # H + 2
GUARD = 99         # tap-shift guard for conv access patterns
NP = HP * WP       # 12740
NT = NP + 2 * GUARD
RST = 112          # f2p row stride (rows 32B-aligned for xbar transpose)
H6 = H + 6
FCH = H6 * RST
FP2 = CH * FCH
RPC = 5            # conv rows per chunk -> 490-wide chunks
NCK = HP // RPC    # 26
CCK = RPC * WP     # 490
YB = 32            # corr row block


def build_nc(n_cores=8):
    import concourse.bacc as bacc
    import concourse.mybir as mybir
    import concourse.tile as tile
    import concourse.bass as bass
    from concourse.masks import make_identity

    f32 = mybir.dt.float32
    bf16 = mybir.dt.bfloat16
    i32 = mybir.dt.int32
    AF = mybir.ActivationFunctionType
    OP = mybir.AluOpType

    nc = bacc.Bacc("TRN2", target_bir_lowering=False, debug=False,
                   num_devices=n_cores)

    dt_in = {}
    dt_in["feat1"] = nc.dram_tensor("feat1", [C, H * W], f32,
                                    kind="ExternalInput")
    dt_in["feat2"] = nc.dram_tensor("feat2", [C, H * W], f32,
                                    kind="ExternalInput")
    convs = [("w1", 49, 128), ("w2", 128, 64), ("w3", 64, 32), ("w4", 32, 2)]
    for i, (wn, K, O) in enumerate(convs):
        dt_in[wn] = nc.dram_tensor(wn, [O, K * 9], f32, kind="ExternalInput")
        dt_in[f"b{i+1}"] = nc.dram_tensor(f"b{i+1}", [1, O], f32,
                                          kind="ExternalInput")
    out_d = nc.dram_tensor("out", [C, H * W], f32, kind="ExternalOutput")
    sd = nc.dram_tensor("sdscratch", [112, 24 * 1536], mybir.dt.bfloat16,
                        kind="Internal")

    def ap(tileap, offset, dims):
        return bass.AP(tensor=tileap.tensor, offset=offset,
                       ap=[list(d) for d in dims])

    def off(c0, ty, tx):
        return GUARD + c0 + (ty - 1) * WP + (tx - 1)

    with tile.TileContext(nc) as tc:
        pp_ctx = tc.tile_pool(name="persist", bufs=1)
        pp = pp_ctx.__enter__()

        # ---------- identities ----------
        id128f = pp.tile([128, 128], f32, tag="idf")
        make_identity(nc, id128f[:])
        idb = pp.tile([128, 128], bf16, tag="idb")
        nc.vector.tensor_copy(idb[:], id128f[:])

        # ---------- biases as [O, 1] ----------
        bv = []
        for i, (wn, K, O) in enumerate(convs):
            b = pp.tile([O, 1], f32, tag=f"bv{i}")
            nc.scalar.dma_start(b[:],
                                dt_in[f"b{i+1}"][:].rearrange("a b -> b a"))
            bv.append(b)

        # ---------- packed conv weights ----------
        w1t = pp.tile([49, 9 * 128], bf16, tag="w1t")
        w2t = pp.tile([128, 9 * 64], bf16, tag="w2t")
        w3p = pp.tile([128, 3 * 32], bf16, tag="w3p")
        w3s = pp.tile([64, 3 * 32], bf16, tag="w3s")
        w4p = pp.tile([96, 3 * 2], bf16, tag="w4p")
        wraw_ctx = tc.tile_pool(name="wraw", bufs=1)
        wrp = wraw_ctx.__enter__()
        psW_ctx = tc.tile_pool(name="psW", bufs=2, space="PSUM")
        psW = psW_ctx.__enter__()
        for i, (wn, K, O) in enumerate(convs):
            wraw = wrp.tile([O, K * 9], f32, tag=f"wr{i}")
            nc.scalar.dma_start(wraw[:], dt_in[wn][:])
            for t in range(9):
                ty, tx = t // 3, t % 3
                wps = psW.tile([128, 128], f32, tag="wps")
                src = ap(wraw[:], t, [[K * 9, O], [9, K]])
                if i == 2 and tx == 2:
                    nc.tensor.transpose(wps[0:64, :O], src, id128f[:O, :O])
                    nc.vector.tensor_copy(w3p[64:128, ty * 32:(ty + 1) * 32],
                                          wps[0:64, :O])
                    continue
                if i == 3:
                    nc.tensor.transpose(wps[0:32, :O], src, id128f[:O, :O])
                    nc.vector.tensor_copy(w4p[tx * 32:(tx + 1) * 32,
                                              ty * 2:(ty + 1) * 2],
                                          wps[0:32, :O])
                    continue
                nc.tensor.transpose(wps[:K, :O], src, id128f[:O, :O])
                if i == 0:
                    dst = w1t[:, t * 128:(t + 1) * 128]
                elif i == 1:
                    dst = w2t[:, t * 64:(t + 1) * 64]
                else:  # i == 2, tx in (0, 1)
                    dst = (w3p[0:64, ty * 32:(ty + 1) * 32] if tx == 1
                           else w3s[:, ty * 32:(ty + 1) * 32])
                nc.scalar.activation(dst, wps[:K, :O], AF.Copy)
        psW_ctx.__exit__(None, None, None)
        wraw_ctx.__exit__(None, None, None)

        # ---------- persistent tensors ----------
        f2p = pp.tile([128, FP2], bf16, tag="f2p")
        # zero only the pad regions (cols 0-2 / 99-101 of every row, plus
        # the 3 pad rows top+bottom per ch); cols 102-111 are never read
        nc.vector.memset(ap(f2p[:], 0, [[FP2, 128], [RST, 2 * H6], [1, 3]]),
                         0.0)
        nc.vector.memset(ap(f2p[:], 99, [[FP2, 128], [RST, 2 * H6], [1, 3]]),
                         0.0)
        for ch in range(CH):
            for r0_ in (0, 131):
                nc.gpsimd.memset(
                    ap(f2p[:], ch * FCH + r0_ * RST,
                       [[FP2, 128], [RST, 3], [1, 102]]), 0.0)
        flow = pp.tile([2, NT], bf16, tag="flow")
        flowT = pp.tile([96, 256], f32, tag="flowT")
        PT = pp.tile([96, 9 * H], bf16, tag="PT")
        xs = pp.tile([96, H], f32, tag="xs")
        ys = pp.tile([96, H], f32, tag="ys")
        xsi = pp.tile([96, H], i32, tag="xsi")
        ysi = pp.tile([96, H], i32, tag="ysi")
        nc.gpsimd.iota(xsi[:], pattern=[[0, H]], base=0, channel_multiplier=1)
        nc.vector.tensor_copy(xs[:], xsi[:])
        nc.gpsimd.iota(ysi[:], pattern=[[1, H]], base=0, channel_multiplier=0)
        nc.vector.tensor_copy(ys[:], ysi[:])

        # f2p load: staged contiguous DMA + convert-scatter (scalar/vector)
        with tc.tile_pool(name="f2st", bufs=3) as f2st:
            for ch in range(CH):
                for q in range(4):
                    stg = f2st.tile([128, 32 * 96], f32, tag="stg",
                                    name="stg")
                    nc.sync.dma_start(
                        stg[:],
                        dt_in["feat2"][ch * 128:(ch + 1) * 128,
                                       q * 32 * 96:(q + 1) * 32 * 96])
                    dst = ap(f2p[:], ch * FCH + (3 + q * 32) * RST + 3,
                             [[FP2, 128], [RST, 32], [1, 96]])
                    if q % 2 == 0:
                        nc.scalar.activation(dst, stg[:], AF.Copy)
                    else:
                        nc.vector.tensor_copy(dst, stg[:])


        def pad_memset(t, O):
            # zero guard head/tail and cols 95-98 of every row (covers the
            # pad columns of all shifted replicas; valid cols rewritten later)
            nc.gpsimd.memset(t[0:O, 0:GUARD + WP + 4], 0.0)
            nc.gpsimd.memset(t[0:O, NT - GUARD - WP - 4:NT], 0.0)
            nc.gpsimd.memset(
                ap(t[:], GUARD + 95, [[NT, O], [WP, 129], [1, 4]]), 0.0)

        # ---------- correlation ----------
        # Two NT-sized activation pools, generation-reused (LIFO-safe):
        # pA: corr -> h2; pB: h1 -> h3.
        pA_ctx = tc.tile_pool(name="pA", bufs=1)
        pA = pA_ctx.__enter__()
        pB_ctx = tc.tile_pool(name="pB", bufs=1)
        pB = pB_ctx.__enter__()
        corr_t = pA.tile([128, NT], bf16, tag="actA", name="corrbuf")
        pad_memset(corr_t, 49)

        # conv1 resources (chunks interleave with corr emission below)
        h1 = pB.tile([128, NT], bf16, tag="actB", name="h1buf")
        pad_memset(h1, 128)
        psC1_ctx = tc.tile_pool(name="psC1", bufs=2, space="PSUM")
        psC1 = psC1_ctx.__enter__()

        def chunk_rows(ck):
            r0 = ck * RPC
            r1 = max(r0, 1)
            r2 = min(r0 + RPC, HP - 1)
            return r0, r1, r2 - r1

        def conv1_chunk(ck):
            c0 = ck * CCK
            r0, r1, nr = chunk_rows(ck)
            ps = psC1.tile([128, CCK], f32, tag="c1", name="c1ps")
            for t in range(9):
                o = off(c0, t // 3, t % 3)
                nc.tensor.matmul(ps[:], w1t[:, t * 128:(t + 1) * 128],
                                 corr_t[0:49, o:o + CCK],
                                 start=(t == 0), stop=(t == 8))
            nc.scalar.activation(
                ap(h1[:], GUARD + r1 * WP + 1, [[NT, 128], [WP, nr], [1, 96]]),
                ap(ps[:], (r1 - r0) * WP + 1, [[CCK, 128], [WP, nr], [1, 96]]),
                AF.Prelu, bias=bv[0][:], alpha=NEG)

        with tc.tile_pool(name="cin", bufs=2) as cin, \
             tc.tile_pool(name="cg", bufs=3) as cg, \
             tc.tile_pool(name="psGa", bufs=2, space="PSUM") as psGa, \
             tc.tile_pool(name="psGb", bufs=2, space="PSUM") as psGb, \
             tc.tile_pool(name="psT", bufs=2, space="PSUM") as psT:
            f1tiles = {}

            def load_f1(qb):
                t = cin.tile([128, CH * YB * 96], bf16, tag="f1blk",
                             name="f1blk")
                for ch in range(CH):
                    nc.gpsimd.dma_start(
                        t[:, ch * YB * 96:(ch + 1) * YB * 96],
                        dt_in["feat1"][ch * 128:(ch + 1) * 128,
                                       qb * YB * 96:(qb + 1) * YB * 96])
                f1tiles[qb] = t

            load_f1(0)
            for y in range(H):
                q, yy = divmod(y, YB)
                if yy == 8 and q + 1 < H // YB:
                    load_f1(q + 1)
                f1blk = f1tiles[q]
                ga = psGa.tile([96, 510], f32, tag="ga")
                gb = psGb.tile([96, 204], f32, tag="gb")
                for ch in range(CH):
                    lhs = f1blk[:, ch * YB * 96 + yy * 96:
                                ch * YB * 96 + (yy + 1) * 96]
                    nc.tensor.matmul(
                        ga[:], lhs,
                        ap(f2p[:], ch * FCH + y * RST,
                           [[FP2, 128], [RST, 5], [1, 102]]),
                        start=(ch == 0), stop=(ch == CH - 1))
                    nc.tensor.matmul(
                        gb[:], lhs,
                        ap(f2p[:], ch * FCH + (y + 5) * RST,
                           [[FP2, 128], [RST, 2], [1, 102]]),
                        start=(ch == 0), stop=(ch == CH - 1))
                gs = cg.tile([96, 714], bf16, tag="gs")
                nc.vector.tensor_copy(gs[:, 0:510], ga[:])
                nc.vector.tensor_copy(gs[:, 510:714], gb[:])
                band = cg.tile([96, 49], bf16, tag="band")
                dmaeng = nc.sync if (y % 2 == 0) else nc.scalar
                dmaeng.dma_start(
                    ap(band[:], 0, [[49, 96], [7, 7], [1, 7]]),
                    ap(gs[:], 0, [[715, 96], [102, 7], [1, 7]]))
                ct = psT.tile([49, 96], bf16, tag="ct")
                nc.tensor.transpose(ct[:], band[:], idb[:96, :96])
                nc.scalar.activation(
                    corr_t[0:49, GUARD + (y + 1) * WP + 1:
                           GUARD + (y + 1) * WP + 1 + 96],
                    ct[:], AF.Prelu, scale=1.0 / C, alpha=NEG)
                if y >= 5 and (y - 5) % RPC == 0 and (y - 5) // RPC < NCK - 1:
                    conv1_chunk((y - 5) // RPC)
        for ck_ in range(NCK - 1 - 4, NCK):
            if ck_ >= (H - 1 - 5) // RPC + 1:
                conv1_chunk(ck_)

        # ---------- convs 2-4 (row-aligned 490 chunks) ----------
        psC_ctx = tc.tile_pool(name="psC", bufs=2, space="PSUM")
        psC = psC_ctx.__enter__()

        # conv2: h1[128] -> h2[64 x2 replicas]
        h2 = pA.tile([128, NT], bf16, tag="actA", name="h2buf")
        pad_memset(h2, 128)
        for ck in range(NCK):
            c0 = ck * CCK
            r0, r1, nr = chunk_rows(ck)
            ps = psC.tile([64, CCK], f32, tag="c2")
            for t in range(9):
                o = off(c0, t // 3, t % 3)
                nc.tensor.matmul(ps[:], w2t[:, t * 64:(t + 1) * 64],
                                 h1[:, o:o + CCK],
                                 start=(t == 0), stop=(t == 8))
            for s in range(2):
                nc.scalar.activation(
                    ap(h2[:], s * 64 * NT + GUARD + r1 * WP + 1 - s,
                       [[NT, 64], [WP, nr], [1, 96]]),
                    ap(ps[:], (r1 - r0) * WP + 1,
                       [[CCK, 64], [WP, nr], [1, 96]]),
                    AF.Prelu, bias=bv[1][:], alpha=NEG)
        # conv3: h2[64x2] -> h3[32 x3 replicas]
        h3 = pB.tile([128, NT], bf16, tag="actB", name="h3buf")
        pad_memset(h3, 96)
        for ck in range(NCK):
            c0 = ck * CCK
            r0, r1, nr = chunk_rows(ck)
            ps = psC.tile([32, CCK], f32, tag="c3")
            for ty in range(3):
                o1_ = off(c0, ty, 1)
                nc.tensor.matmul(ps[:], w3p[:, ty * 32:(ty + 1) * 32],
                                 h2[:, o1_:o1_ + CCK],
                                 start=(ty == 0), stop=False)
                o0_ = off(c0, ty, 0)
                nc.tensor.matmul(ps[:], w3s[:, ty * 32:(ty + 1) * 32],
                                 h2[0:64, o0_:o0_ + CCK],
                                 start=False, stop=(ty == 2))
            for s in range(3):
                nc.scalar.activation(
                    ap(h3[:], s * 32 * NT + GUARD + r1 * WP + 1 - s,
                       [[NT, 32], [WP, nr], [1, 96]]),
                    ap(ps[:], (r1 - r0) * WP + 1,
                       [[CCK, 32], [WP, nr], [1, 96]]),
                    AF.Prelu, bias=bv[2][:], alpha=NEG)
        # conv4: h3[32x3] -> flow[2] (bias, no lrelu)
        for ck in range(NCK):
            c0 = ck * CCK
            r0, r1, nr = chunk_rows(ck)
            ps = psC.tile([2, CCK], f32, tag="c4")
            for ty in range(3):
                o0_ = off(c0, ty, 0)
                nc.tensor.matmul(ps[:], w4p[:, ty * 2:(ty + 1) * 2],
                                 h3[0:96, o0_:o0_ + CCK],
                                 start=(ty == 0), stop=(ty == 2))
            nc.scalar.activation(
                ap(flow[:], GUARD + r1 * WP + 1, [[NT, 2], [WP, nr], [1, 96]]),
                ap(ps[:], (r1 - r0) * WP + 1, [[CCK, 2], [WP, nr], [1, 96]]),
                AF.Identity, bias=bv[3][:])
        pB_ctx.__exit__(None, None, None)
        psC_ctx.__exit__(None, None, None)
        psC1_ctx.__exit__(None, None, None)

        # ---------- flow -> flowT -> coefficient planes ----------
        with tc.tile_pool(name="cf", bufs=2) as cf, \
             tc.tile_pool(name="psF", bufs=2, space="PSUM") as psF:
            for q in range(4):
                fp = cf.tile([64, 96], bf16, tag="fp")
                nc.sync.dma_start(
                    ap(fp[:], 0, [[96, 64], [1, 96]]),
                    ap(flow[:], GUARD + (q * 32 + 1) * WP + 1,
                       [[NT, 2], [WP, 32], [1, 96]]))
                pf = psF.tile([96, 64], bf16, tag="pf")
                nc.tensor.transpose(pf[:], fp[:], idb[:64, :64])
                nc.scalar.activation(flowT[:, q * 64:(q + 1) * 64], pf[:],
                                     AF.Copy)

        with tc.tile_pool(name="cp", bufs=1) as cp:
            fxv = ap(flowT[:], 0, [[256, 96], [64, 4], [1, 32]])
            fyv = ap(flowT[:], 32, [[256, 96], [64, 4], [1, 32]])

            def coeffs(fv, base, lim, pfx):
                p = cp.tile([96, H], f32, tag=pfx + "p")
                nc.vector.tensor_tensor(out=p[:], in0=fv, in1=base[:],
                                        op=OP.add)
                nc.vector.tensor_scalar(out=p[:], in0=p[:], scalar1=0.0,
                                        scalar2=float(lim), op0=OP.max,
                                        op1=OP.min)
                pi = cp.tile([96, H], i32, tag=pfx + "pi")
                nc.vector.tensor_copy(pi[:], p[:])
                pf_ = cp.tile([96, H], f32, tag=pfx + "pf")
                nc.vector.tensor_copy(pf_[:], pi[:])
                gt = cp.tile([96, H], f32, tag=pfx + "gt")
                nc.vector.tensor_tensor(out=gt[:], in0=pf_[:], in1=p[:],
                                        op=OP.is_gt)
                nc.vector.tensor_tensor(out=pf_[:], in0=pf_[:], in1=gt[:],
                                        op=OP.subtract)  # floor(p)
                w_ = cp.tile([96, H], f32, tag=pfx + "w")
                nc.vector.tensor_tensor(out=w_[:], in0=p[:], in1=pf_[:],
                                        op=OP.subtract)
                a = cp.tile([96, H], f32, tag=pfx + "a")
                nc.vector.tensor_tensor(out=a[:], in0=pf_[:], in1=base[:],
                                        op=OP.subtract)  # in {-1, 0}
                cm = cp.tile([96, H], f32, tag=pfx + "cm")
                t1 = cp.tile([96, H], f32, tag=pfx + "t1")
                nc.vector.tensor_scalar(out=t1[:], in0=w_[:], scalar1=-1.0,
                                        scalar2=1.0, op0=OP.mult, op1=OP.add)
                nc.vector.tensor_scalar(out=cm[:], in0=a[:], scalar1=-1.0,
                                        scalar2=None, op0=OP.mult)
                nc.vector.tensor_tensor(out=cm[:], in0=cm[:], in1=t1[:],
                                        op=OP.mult)
                cpl = cp.tile([96, H], f32, tag=pfx + "cp")
                nc.vector.tensor_scalar(out=cpl[:], in0=a[:], scalar1=1.0,
                                        scalar2=None, op0=OP.add)
                nc.vector.tensor_tensor(out=cpl[:], in0=cpl[:], in1=w_[:],
                                        op=OP.mult)
                c0_ = cp.tile([96, H], f32, tag=pfx + "c0")
                nc.vector.tensor_tensor(out=c0_[:], in0=cm[:], in1=cpl[:],
                                        op=OP.add)
                nc.vector.tensor_scalar(out=c0_[:], in0=c0_[:], scalar1=-1.0,
                                        scalar2=1.0, op0=OP.mult, op1=OP.add)
                return cm, c0_, cpl

            cxs = coeffs(fxv, xs, W - 1, "x")
            cys = coeffs(fyv, ys, H - 1, "y")
            for ty in range(3):
                for s in range(3):
                    nc.vector.tensor_tensor(
                        out=PT[:, (ty * 3 + s) * H:(ty * 3 + s + 1) * H],
                        in0=cys[ty][:], in1=cxs[s][:], op=OP.mult)

        # DRAM-roundtrip S-build scratch: zero it once (sync queue FIFO
        # orders this before the diagonal writes below).
        ztile = pp.tile([112, 1536], bf16, tag="ztile")
        nc.vector.memset(ztile[:], 0.0)
        nc.sync.dma_start(
            bass.AP(tensor=sd[:].tensor, offset=0,
                    ap=[[24 * 1536, 112], [1536, 24], [1, 1536]]),
            ap(ztile[:], 0, [[1536, 112], [0, 24], [1, 1536]]))

        # ---------- warp (TensorEngine bilinear via banded S matmuls) ----
        pS_ctx = tc.tile_pool(name="pS", bufs=3)
        pS = pS_ctx.__enter__()
        pwp_ctx = tc.tile_pool(name="pwp", bufs=3)
        pwp = pwp_ctx.__enter__()
        pst_ctx = tc.tile_pool(name="pst", bufs=2)
        pst = pst_ctx.__enter__()
        psO_ctx = tc.tile_pool(name="psO", bufs=6, space="PSUM")
        psO = psO_ctx.__enter__()

        S_of_block = {}

        def build_S(q):
            tiles = []
            for ty in range(3):
                blk = ty * 8 + q
                # diagonal write: sd[dst+2+s, blk*1536 + dst*16 + y]
                #   = PT[dst, (ty*3+s)*H + q*16 + y]
                nc.sync.dma_start(
                    bass.AP(tensor=sd[:].tensor,
                            offset=2 * 36864 + blk * 1536,
                            ap=[[36880, 96], [36864, 3], [1, 16]]),
                    ap(PT[:], ty * 3 * H + q * 16,
                       [[9 * H, 96], [H, 3], [1, 16]]))
                st = pS.tile([112, 16 * 96], bf16, tag=f"S{ty}", name="st")
                nc.sync.dma_start(
                    st[:],
                    bass.AP(tensor=sd[:].tensor, offset=blk * 1536,
                            ap=[[36864, 112], [1, 1536]]))
                tiles.append(st)
            S_of_block[q] = tiles

        psT2_ctx = tc.tile_pool(name="psT2", bufs=2, space="PSUM")
        psT2 = psT2_ctx.__enter__()
        build_S(0)
        psO_by_y = {}
        stag = {}
        for yp in range(H):
            if yp % 16 == 4 and yp // 16 + 1 < 8:
                build_S(yp // 16 + 1)
            mst = pwp.tile([128, 256], bf16, tag="mst")
            for ch in range(CH):
                tp = psT2.tile([102, 128], bf16, tag="tp", name="tp")
                nc.tensor.transpose(
                    tp[:],
                    f2p[:, ch * FCH + (yp + 3) * RST:
                        ch * FCH + (yp + 3) * RST + 102],
                    idb[:])
                nc.scalar.activation(mst[0:102, ch * 128:(ch + 1) * 128],
                                     tp[:], AF.Copy)
            for ch in range(CH):
                lhsT = mst[0:100, ch * 128:(ch + 1) * 128]
                for ty in range(3):
                    yt = yp + 1 - ty
                    if not (0 <= yt < H):
                        continue
                    key = (yt, ch)
                    if key not in psO_by_y:
                        psO_by_y[key] = psO.tile([128, 96], f32, tag="po", name="po")
                    qt = yt // 16
                    st = S_of_block[qt][ty]
                    start = (ty == 0) or (yt == 0 and ty == 1)
                    stop = (ty == 2) or (yt == H - 1 and ty == 1)
                    nc.tensor.matmul(
                        psO_by_y[key][:], lhsT,
                        ap(st[:], yt - qt * 16, [[1536, 100], [16, 96]]),
                        start=start, stop=stop)
            done = []
            if yp >= 1:
                done.append(yp - 1)
            if yp == H - 1:
                done.append(H - 1)
            for yt in done:
                g = yt // 16
                if yt % 16 == 0:
                    stag[g] = [pst.tile([128, 16 * 96], f32, tag=f"st{ch}",
                                        name=f"st{ch}")
                               for ch in range(2)]
                for ch in range(2):
                    po = psO_by_y.pop((yt, ch))
                    nc.vector.tensor_copy(
                        stag[g][ch][:, (yt % 16) * 96:(yt % 16 + 1) * 96],
                        po[:])
                if yt % 16 == 15:
                    for ch in range(2):
                        nc.sync.dma_start(
                            out_d[ch * 128:(ch + 1) * 128,
                                  g * 16 * 96:(g + 1) * 16 * 96],
                            stag[g][ch][:])

        psT2_ctx.__exit__(None, None, None)
        psO_ctx.__exit__(None, None, None)
        pst_ctx.__exit__(None, None, None)
        pwp_ctx.__exit__(None, None, None)
        pS_ctx.__exit__(None, None, None)
        pA_ctx.__exit__(None, None, None)
        pp_ctx.__exit__(None, None, None)

    import os
    if not os.environ.get("AFWM_SKIP_COMPILE"):
        nc.compile()
    return nc


_CACHE = {}


def _get_nc():
    if "nc" not in _CACHE:
        _CACHE["nc"] = build_nc(n_cores=8)
    return _CACHE["nc"]


def _np_reference(feat1, feat2, w1, b1, w2, b2, w3, b3, w4, b4, stride):
    """Safety-net numpy fallback for unexpected shapes."""
    def lrelu(x):
        return np.where(x > 0, x, NEG * x)

    def conv3(x, w, b):
        Bb, Ci, Hh, Ww = x.shape
        O = w.shape[0]
        xp = np.pad(x, ((0, 0), (0, 0), (1, 1), (1, 1)))
        y = np.zeros((Bb, O, Hh, Ww), np.float32)
        for ky in range(3):
            for kx in range(3):
                patch = xp[:, :, ky:ky + Hh, kx:kx + Ww]
                y += np.einsum("oc,bchw->bohw", w[:, :, ky, kx], patch)
        return y + b[None, :, None, None]

    B, Cc, Hh, Ww = feat1.shape
    pad = 3 * stride
    f1s = feat1[:, :, ::stride, ::stride]
    f2p = np.pad(feat2, ((0, 0), (0, 0), (pad, pad), (pad, pad)))
    outs = []
    for dy in range(7):
        for dx in range(7):
            sl = f2p[:, :, dy * stride:dy * stride + Hh:stride,
                     dx * stride:dx * stride + Ww:stride]
            outs.append(np.sum(f1s * sl, axis=1))
    corr = lrelu(np.stack(outs, axis=1) / Cc)
    h = lrelu(conv3(corr, w1, b1))
    h = lrelu(conv3(h, w2, b2))
    h = lrelu(conv3(h, w3, b3))
    fl = conv3(h, w4, b4)
    gx = np.clip(np.arange(Ww, dtype=np.float32)[None, None] + fl[:, 0], 0,
                 Ww - 1)
    gy = np.clip(np.arange(Hh, dtype=np.float32)[None, :, None] + fl[:, 1], 0,
                 Hh - 1)
    x0 = np.floor(gx).astype(np.int64)
    y0 = np.floor(gy).astype(np.int64)
    x1 = np.minimum(x0 + 1, Ww - 1)
    y1 = np.minimum(y0 + 1, Hh - 1)
    wx = (gx - x0)[:, None]
    wy = (gy - y0)[:, None]
    ff = feat2.reshape(B, Cc, Hh * Ww)

    def g(yy, xx):
        idx = (yy * Ww + xx).reshape(B, 1, Hh * Ww)
        return np.take_along_axis(ff, np.broadcast_to(idx, ff.shape),
                                  axis=2).reshape(B, Cc, Hh, Ww)

    return (g(y0, x0) * (1 - wx) * (1 - wy) + g(y0, x1) * wx * (1 - wy)
            + g(y1, x0) * (1 - wx) * wy
            + g(y1, x1) * wx * wy).astype(np.float32)


def kernel(feat1, feat2, w1, b1, w2, b2, w3, b3, w4, b4, stride=1, **_):
    from concourse.bass_utils import run_bass_kernel_spmd

    stride = int(stride)
    if stride != 1 or feat1.shape != (8, 256, 128, 96):
        return _np_reference(feat1, feat2, w1, b1, w2, b2, w3, b3, w4, b4,
                             stride)

    nc = _get_nc()
    B = feat1.shape[0]
    in_maps = []
    for b in range(B):
        m = {
            "feat1": np.ascontiguousarray(feat1[b].reshape(256, -1),
                                          np.float32),
            "feat2": np.ascontiguousarray(feat2[b].reshape(256, -1),
                                          np.float32),
        }
        for i, wv in enumerate((w1, w2, w3, w4)):
            m[f"w{i+1}"] = np.ascontiguousarray(
                wv.reshape(wv.shape[0], -1), np.float32)
        for i, bvv in enumerate((b1, b2, b3, b4)):
            m[f"b{i+1}"] = np.ascontiguousarray(
                bvv.reshape(1, -1), np.float32)
        in_maps.append(m)
    res = run_bass_kernel_spmd(nc, in_maps, core_ids=list(range(8)))
    out = np.stack([res.results[b]["out"].reshape(256, 128, 96)
                    for b in range(B)])
    return np.ascontiguousarray(out, np.float32)


if __name__ == "__main__":
    rng = np.random.default_rng(0)
    ins = dict(
        feat1=rng.standard_normal((8, 256, 128, 96), dtype=np.float32),
        feat2=rng.standard_normal((8, 256, 128, 96), dtype=np.float32),
        w1=(0.05 * rng.standard_normal((128, 49, 3, 3))).astype(np.float32),
        b1=np.zeros(128, np.float32),
        w2=(0.05 * rng.standard_normal((64, 128, 3, 3))).astype(np.float32),
        b2=np.zeros(64, np.float32),
        w3=(0.05 * rng.standard_normal((32, 64, 3, 3))).astype(np.float32),
        b3=np.zeros(32, np.float32),
        w4=(0.05 * rng.standard_normal((2, 32, 3, 3))).astype(np.float32),
        b4=np.zeros(2, np.float32),
        stride=1,
    )
    t0 = time.perf_counter()
    out = kernel(**ins)
    print("out", out.shape, float(np.abs(out).max()),
          f"{time.perf_counter() - t0:.1f}s")
